# revision 8
# baseline (speedup 1.0000x reference)
"""ALNN layer on 8 TRN2 NeuronCores (Bass/Tile, SPMD — no collectives).

Math (per reference):
  ref_r = linspace(0, 48, 64);  a_r = relu(alpha_r)
  e[b,r,l,d]  = exp(-a_r * |T[b,l,d] - ref_r|)
  p[b,r,l,d]  = w0*X + w1*relu(X)*e + w2*M + w3*DT + w4*P + 5*b_t[r,l,d]
  h           = relu(p)
  out[b,r,d]  = relu( sum_l w_v[r,l,d]*h + 128*b_v[r,d] )

Design v2 (DVE-roofline focused):
- Shard R=64 across the 8 cores (8 r each); inputs replicated; host
  concatenates the per-core [B, 8, D] outputs. No cross-core traffic.
- Layout: partition = L (=128), free = (r-pair, b, d). Per pair the 12
  irreducible DVE element-passes are packed into 11 instructions, with the
  4 channel muls done as two 2-channel-wide ops (bf16 2x_1p mode) reading a
  host-packed C5 = [XP, X, M, DT, P] tile, and the add tree packed as
  [aXM]+[aDP] -> s12, s12[0]+s12[1] -> s3.
- DMA is ~12 large transfers ordered so ACT (T, RN, AN) and DVE (XP, W0)
  start by ~4us; weight packs are one DMA per pair (prefetched bufs=2).
- ACT (ScalarE): per-r dist=Abs(T-ref), e=Exp(-a*dist) (f32 dist), relu(p),
  and the per-pair psum epilogue relu. Pair jj+1's dist/exp are issued
  before pair jj's relu so the ACT queue never blocks DVE.
- TensorE: per (pair, b-chunk) PSUM accumulation groups (4 banks per pair,
  2 pairs in flight over the 8 banks): bias open via identity-rhs matmul,
  then per-r ones-column lhsT matmuls sum wv*h over l. Each pair's group
  closes right after its wh, so relu(psum) + output DMA overlap the next
  pair's compute (no serial tail).
"""
import sys

import numpy as np

if "/opt/trn_rl_repo" not in sys.path:
    sys.path.insert(0, "/opt/trn_rl_repo")

import ml_dtypes

from concourse import bacc, mybir
import concourse.tile as tile
from concourse.bass_utils import run_bass_kernel_spmd

BF16 = ml_dtypes.bfloat16
B, L, D = 32, 128, 48
R = 64
RL = R // 8  # r per core
NP = RL // 2  # r-pairs per core
INIT_TIME, MAX_TS = 0.0, 48.0

_CACHE = {}


def _build():
    nc = bacc.Bacc("TRN2", target_bir_lowering=False, debug=False, num_devices=8)
    f32, bf16 = mybir.dt.float32, mybir.dt.bfloat16
    AF = mybir.ActivationFunctionType

    # DRAM parameters (per-core shards / replicas)
    dTt = nc.dram_tensor("Tt", [L, B, D], f32, kind="ExternalInput").ap()
    # C5 channels: (XP, X, M, DT, P)
    dC5 = nc.dram_tensor("C5", [L, 5, B, D], bf16, kind="ExternalInput").ap()
    # W channels: (w1, w0, w2, w3, w4, 5*b_t, w_v) per r-pair
    dW = nc.dram_tensor("W", [NP, L, 7, 2, 1, D], bf16, kind="ExternalInput").ap()
    dRN = nc.dram_tensor("RN", [L, RL], f32, kind="ExternalInput").ap()
    dAN = nc.dram_tensor("AN", [L, RL], f32, kind="ExternalInput").ap()
    dBV = nc.dram_tensor("BVl", [D, RL], bf16, kind="ExternalInput").ap()
    dID = nc.dram_tensor("ID48", [D, D], bf16, kind="ExternalInput").ap()
    dOH = nc.dram_tensor("OHP", [L, 2, 2], bf16, kind="ExternalInput").ap()
    dOUT = nc.dram_tensor("out", [B, RL, D], f32, kind="ExternalOutput").ap()

    NCH = 4  # psum b-chunks per pair (2*8*48 = 768 f32 < ... 8*48=384/bank)
    BC = B // NCH  # 8 b per chunk

    with tile.TileContext(nc) as tc:
        with (
            tc.tile_pool(name="const", bufs=1) as cpool,
            tc.tile_pool(name="work", bufs=2) as wpool,
            tc.tile_pool(name="psum", bufs=1, space="PSUM") as ppool,
            tc.tile_pool(name="outp", bufs=1) as opool,
        ):
            # ---- DMA startup plan: two queues in parallel.
            # sync: ACT-chain gates (RN, AN, T) then matmul consts.
            # gpsimd: DVE gates (XP, W0, C5 channels, W1).
            tRN = cpool.tile([L, RL], f32, tag="RN")
            nc.sync.dma_start(tRN[:], dRN)
            tAN = cpool.tile([L, RL], f32, tag="AN")
            nc.sync.dma_start(tAN[:], dAN)
            tT = cpool.tile([L, B, D], f32, tag="T")
            nc.sync.dma_start(tT[:], dTt)
            # separate tiles per channel group so readers dep on exactly
            # their own DMA (one tC5 tile made every reader wait for the
            # LAST C5 slice to land)
            tXP = cpool.tile([L, 1, B, D], bf16, tag="XP")
            nc.gpsimd.dma_start(tXP[:], dC5[:, 0:1])  # XP: gates q-mul
            wts = [
                wpool.tile([L, 7, 2, 1, D], bf16, tag="wt", name=f"wt{j}", bufs=2)
                for j in range(NP)
            ]
            nc.gpsimd.dma_start(wts[0][:], dW[0])
            tC5a = cpool.tile([L, 2, B, D], bf16, tag="C5a")
            nc.gpsimd.dma_start(tC5a[:], dC5[:, 1:3])  # X, M: gate mul2a
            tC5b = cpool.tile([L, 2, B, D], bf16, tag="C5b")
            nc.gpsimd.dma_start(tC5b[:], dC5[:, 3:5])  # DT, P
            nc.gpsimd.dma_start(wts[1][:], dW[1])
            tBV = cpool.tile([D, RL], bf16, tag="BV")
            nc.sync.dma_start(tBV[:], dBV)
            tID = cpool.tile([D, D], bf16, tag="ID")
            nc.sync.dma_start(tID[:], dID)
            tOH = cpool.tile([L, 2, 2], bf16, tag="OH")
            nc.sync.dma_start(tOH[:], dOH)

            # q = XP*w1 and t = q*e run on GpSimd (Pool), one pair ahead of
            # the DVE pipeline: they only depend on DMAs and the ACT exp
            # chain, so Pool's ~4x-slower software tensor ops stay off the
            # critical path while freeing ~3.4us of DVE time per pair.
            qs, ts = {}, {}

            def issue_qt(jj):
                wt = wts[jj]
                q = wpool.tile([L, 2, B, D], bf16, tag="q", name=f"q{jj}", bufs=2)
                nc.gpsimd.tensor_mul(
                    q[:],
                    tC5[:, 0:1].to_broadcast((L, 2, B, D)),
                    wt[:, 0, :, :, :].to_broadcast((L, 2, B, D)),
                )
                t = wpool.tile([L, 2, B, D], bf16, tag="t", name=f"t{jj}", bufs=2)
                nc.gpsimd.tensor_mul(t[:], q[:], ebfs[jj][:])
                qs[jj], ts[jj] = q, t

            # one psum tile = all 8 banks; pair jj uses banks 4*(jj%2)..+4
            pp = ppool.tile([2, 8, 512], mybir.dt.float32, tag="ps", name="pp")
            ppv = lambda jj, c: pp[:, 4 * (jj % 2) + c, : BC * D].rearrange(
                "p (b d) -> p b d", b=BC
            )

            dOUTt = dOUT.transpose([1, 0, 2])  # [RL, B, D]

            S5 = lambda k: (L, k, 2, B, D)
            dists = [None, None]
            ebfs = {}

            def issue_dist_exp(jj):
                ebf = wpool.tile([L, 2, B, D], bf16, tag="ebf", name=f"ebf{jj}", bufs=2)
                ebfs[jj] = ebf
                for rr in range(2):
                    j = 2 * jj + rr
                    dist = wpool.tile(
                        [L, B, D], f32, tag="dist", name=f"dist{j}", bufs=2
                    )
                    nc.scalar.activation(
                        dist[:], tT[:], AF.Abs, bias=tRN[:, j : j + 1]
                    )
                    nc.scalar.activation(
                        ebf[:, rr], dist[:], AF.Exp, scale=tAN[:, j : j + 1]
                    )

            issue_dist_exp(0)
            issue_qt(0)

            for jj in range(NP):
                wt = wts[jj]
                last = jj == NP - 1
                if jj + 2 < NP:
                    nc.sync.dma_start(wts[jj + 2][:], dW[jj + 2])
                if jj + 1 < NP:
                    issue_dist_exp(jj + 1)
                    issue_qt(jj + 1)
                ebfs.pop(jj)
                q, t = qs.pop(jj), ts.pop(jj)

                # psum groups open: bias = 128*b_v via identity-rhs matmul
                for c in range(NCH):
                    nc.tensor.matmul(
                        ppv(jj, c),
                        tBV[:, 2 * jj : 2 * jj + 2],
                        tID[:, None, :].to_broadcast((D, BC, D)),
                        start=True,
                        stop=False,
                    )

                # ---- DVE: 8-9 instructions, 9.5 element-passes
                a2a = wpool.tile([L, 2, 2, B, D], bf16, tag="a2a", bufs=1)
                nc.vector.tensor_mul(
                    a2a[:],
                    tC5[:, 1:3, None].to_broadcast(S5(2)),
                    wt[:, 1:3].to_broadcast(S5(2)),
                )
                a2b = wpool.tile([L, 2, 2, B, D], bf16, tag="a2b", bufs=1)
                nc.vector.tensor_mul(
                    a2b[:],
                    tC5[:, 3:5, None].to_broadcast(S5(2)),
                    wt[:, 3:5].to_broadcast(S5(2)),
                )
                s12 = wpool.tile([L, 2, 2, B, D], bf16, tag="s12", bufs=1)
                nc.vector.tensor_add(s12[:], a2a[:], a2b[:])
                s3 = wpool.tile([L, 2, B, D], bf16, tag="s3", bufs=2)
                nc.vector.tensor_add(s3[:], s12[:, 0], s12[:, 1])
                tb = wpool.tile([L, 2, B, D], bf16, tag="tb", bufs=2)
                nc.vector.tensor_add(
                    tb[:], t[:], wt[:, 5].to_broadcast((L, 2, B, D))
                )
                p = wpool.tile([L, 2, B, D], bf16, tag="p", bufs=2)
                h = wpool.tile([L, 2, B, D], bf16, tag="h", bufs=2)
                wh = wpool.tile([L, 2, B, D], bf16, tag="wh", bufs=2)
                rsl = [slice(0, 2)] if not last else [slice(0, 1), slice(1, 2)]
                for rs in rsl:
                    # last pair: split the p->relu->wh tail per r so ACT/PE
                    # overlap DVE instead of serializing after it
                    nc.vector.tensor_add(p[:, rs], s3[:, rs], tb[:, rs])
                    nc.scalar.activation(h[:, rs], p[:, rs], AF.Relu)
                for rr in range(2):
                    nc.vector.tensor_mul(
                        wh[:, rr],
                        h[:, rr],
                        wt[:, 6, rr].to_broadcast((L, B, D)),
                    )
                    for c in range(NCH):
                        nc.tensor.matmul(
                            ppv(jj, c),
                            tOH[:, rr],
                            wh[:, rr, c * BC : (c + 1) * BC, :],
                            start=False,
                            stop=(rr == 1),
                        )

                # epilogue for this pair: relu(psum) -> sbuf, DMA out
                outf = opool.tile(
                    [2, B, D], mybir.dt.float32, tag="outf", name=f"outf{jj}", bufs=2
                )
                h4 = 4 * (jj % 2)
                nc.scalar.activation(
                    outf.rearrange("p (c b) d -> p c b d", c=NCH),
                    pp[:, h4 : h4 + 4, : BC * D].rearrange(
                        "p c (b d) -> p c b d", b=BC
                    ),
                    AF.Relu,
                )
                nc.sync.dma_start(dOUTt[2 * jj : 2 * jj + 2], outf[:])

    nc.compile()
    return nc


def _prep(X, T, M, DT, P, alpha, w_t, b_t, w_v, b_v):
    """Host-side shard prep: returns in_maps for the 8 cores."""
    X, T, M, DT, P, alpha, w_t, b_t, w_v, b_v = (
        np.asarray(a) for a in (X, T, M, DT, P, alpha, w_t, b_t, w_v, b_v)
    )
    refs = np.linspace(INIT_TIME, MAX_TS, R, dtype=np.float32)
    arelu = np.maximum(alpha.reshape(R).astype(np.float32), 0.0)

    Tt = np.ascontiguousarray(T.transpose(1, 0, 2)).astype(np.float32)
    Xb = X.transpose(1, 0, 2).astype(BF16)
    c5 = np.ascontiguousarray(
        np.stack(
            [
                np.maximum(Xb, 0),
                Xb,
                M.transpose(1, 0, 2).astype(BF16),
                DT.transpose(1, 0, 2).astype(BF16),
                P.transpose(1, 0, 2).astype(BF16),
            ],
            axis=1,
        )
    )  # [L, 5, B, D]
    id48 = np.eye(D, dtype=np.float32).astype(BF16)
    ohp = np.zeros((L, 2, 2), dtype=np.float32)
    ohp[:, 0, 0] = 1.0
    ohp[:, 1, 1] = 1.0
    ohp = ohp.astype(BF16)

    # W[pair, l, k, rr, 1, d]: channels (w1, w0, w2, w3, w4, 5*b_t, w_v)
    wk_full = np.concatenate(
        [
            w_t[..., 1:2],
            w_t[..., 0:1],
            w_t[..., 2:5],
            5.0 * b_t,
            w_v[..., None],
        ],
        axis=3,
    )  # [R, L, D, 7]
    in_maps = []
    for i in range(8):
        r0 = i * RL
        wx = wk_full[r0 : r0 + RL].transpose(1, 3, 0, 2)  # [L, 7, RL, D]
        wx = wx.reshape(L, 7, NP, 2, D).transpose(2, 0, 1, 3, 4)  # [NP, L, 7, 2, D]
        wx = np.ascontiguousarray(wx[:, :, :, :, None, :]).astype(BF16)
        rn = np.broadcast_to(-refs[r0 : r0 + RL], (L, RL)).astype(np.float32)
        an = np.broadcast_to(-arelu[r0 : r0 + RL], (L, RL)).astype(np.float32)
        bvl = np.ascontiguousarray(
            (128.0 * b_v[r0 : r0 + RL, 0, :]).T
        ).astype(BF16)  # [D, RL]
        in_maps.append(
            {
                "Tt": Tt,
                "C5": c5,
                "W": wx,
                "RN": np.ascontiguousarray(rn),
                "AN": np.ascontiguousarray(an),
                "BVl": bvl,
                "ID48": id48,
                "OHP": ohp,
            }
        )
    return in_maps


def run(trace=False, **inputs):
    if "nc" not in _CACHE:
        _CACHE["nc"] = _build()
    nc = _CACHE["nc"]
    in_maps = _prep(**inputs)
    res = run_bass_kernel_spmd(nc, in_maps, core_ids=list(range(8)), trace=trace)
    out = np.empty((B, R, D), dtype=np.float32)
    for i in range(8):
        out[:, i * RL : (i + 1) * RL, :] = res.results[i]["out"]
    return out, res


def kernel(**inputs) -> np.ndarray:
    out, _ = run(trace=False, **inputs)
    return out


# revision 15
# speedup vs baseline: 1.2903x; 1.2903x over previous
"""ALNN layer on 8 TRN2 NeuronCores (Bass/Tile, SPMD — no collectives).

Math (per reference):
  ref_r = linspace(0, 48, 64);  a_r = relu(alpha_r)
  e[b,r,l,d]  = exp(-a_r * |T[b,l,d] - ref_r|)
  p[b,r,l,d]  = w0*X + w1*relu(X)*e + w2*M + w3*DT + w4*P + 5*b_t[r,l,d]
  h           = relu(p)
  out[b,r,d]  = relu( sum_l w_v[r,l,d]*h + 128*b_v[r,d] )

Design v2 (DVE-roofline focused):
- Shard R=64 across the 8 cores (8 r each); inputs replicated; host
  concatenates the per-core [B, 8, D] outputs. No cross-core traffic.
- Layout: partition = L (=128), free = (r-pair, b, d). Per pair the 12
  irreducible DVE element-passes are packed into 11 instructions, with the
  4 channel muls done as two 2-channel-wide ops (bf16 2x_1p mode) reading a
  host-packed C5 = [XP, X, M, DT, P] tile, and the add tree packed as
  [aXM]+[aDP] -> s12, s12[0]+s12[1] -> s3.
- DMA is ~12 large transfers ordered so ACT (T, RN, AN) and DVE (XP, W0)
  start by ~4us; weight packs are one DMA per pair (prefetched bufs=2).
- ACT (ScalarE): per-r dist=Abs(T-ref), e=Exp(-a*dist) (f32 dist), relu(p),
  and the per-pair psum epilogue relu. Pair jj+1's dist/exp are issued
  before pair jj's relu so the ACT queue never blocks DVE.
- TensorE: per (pair, b-chunk) PSUM accumulation groups (4 banks per pair,
  2 pairs in flight over the 8 banks): bias open via identity-rhs matmul,
  then per-r ones-column lhsT matmuls sum wv*h over l. Each pair's group
  closes right after its wh, so relu(psum) + output DMA overlap the next
  pair's compute (no serial tail).
"""
import sys

import numpy as np

if "/opt/trn_rl_repo" not in sys.path:
    sys.path.insert(0, "/opt/trn_rl_repo")

import ml_dtypes

from concourse import bacc, mybir
import concourse.tile as tile
from concourse.bass_utils import run_bass_kernel_spmd

BF16 = ml_dtypes.bfloat16
B, L, D = 32, 128, 48
R = 64
RL = R // 8  # r per core
NP = RL // 2  # r-pairs per core
INIT_TIME, MAX_TS = 0.0, 48.0

_CACHE = {}


def _build():
    nc = bacc.Bacc("TRN2", target_bir_lowering=False, debug=False, num_devices=8)
    f32, bf16 = mybir.dt.float32, mybir.dt.bfloat16
    AF = mybir.ActivationFunctionType

    # DRAM parameters (per-core shards / replicas)
    dTt = nc.dram_tensor("Tt", [L, B, D], f32, kind="ExternalInput").ap()
    # C5 channels: (XP, X, M, DT, P)
    dC5 = nc.dram_tensor("C5", [L, 5, B, D], bf16, kind="ExternalInput").ap()
    # W channels: (w1, w0, w2, w3, w4, 5*b_t, w_v) per r-pair
    dW = nc.dram_tensor("W", [NP, L, 7, 2, 1, D], bf16, kind="ExternalInput").ap()
    # RA[:, 0] = -refs (dist bias), RA[:, 1] = -relu(alpha) (exp scale)
    dRA = nc.dram_tensor("RA", [L, 2, RL], f32, kind="ExternalInput").ap()
    dBV = nc.dram_tensor("BVl", [D, RL], bf16, kind="ExternalInput").ap()
    dID = nc.dram_tensor("ID48", [D, D], bf16, kind="ExternalInput").ap()
    dOH = nc.dram_tensor("OHP", [L, 2, 2], bf16, kind="ExternalInput").ap()
    dOUT = nc.dram_tensor("out", [B, RL, D], f32, kind="ExternalOutput").ap()

    NCH = 4  # psum b-chunks per pair (2*8*48 = 768 f32 < ... 8*48=384/bank)
    BC = B // NCH  # 8 b per chunk

    with tile.TileContext(nc) as tc:
        with (
            tc.tile_pool(name="const", bufs=1) as cpool,
            tc.tile_pool(name="work", bufs=2) as wpool,
            tc.tile_pool(name="psum", bufs=1, space="PSUM") as ppool,
            tc.tile_pool(name="outp", bufs=1) as opool,
        ):
            # ---- DMA startup plan: two queues in parallel, exact-dep tiles.
            # sync: ACT-chain gates (RA, T) then matmul consts + W2/W3.
            # gpsimd: DVE gates (W0, XP, C5a, C5b, W1).
            tRA = cpool.tile([L, 2, RL], f32, tag="RA")
            nc.sync.dma_start(tRA[:], dRA)
            tT = cpool.tile([L, B, D], f32, tag="T")
            nc.sync.dma_start(tT[:], dTt)
            wts = [
                wpool.tile([L, 7, 2, 1, D], bf16, tag="wt", name=f"wt{j}", bufs=2)
                for j in range(NP)
            ]
            nc.gpsimd.dma_start(wts[0][:], dW[0])
            tXP = cpool.tile([L, 1, B, D], bf16, tag="XP")
            nc.gpsimd.dma_start(tXP[:], dC5[:, 0:1])  # XP: gates q-mul
            tC5a = cpool.tile([L, 2, B, D], bf16, tag="C5a")
            nc.gpsimd.dma_start(tC5a[:], dC5[:, 1:3])  # X, M: gate mul2a
            tC5b = cpool.tile([L, 2, B, D], bf16, tag="C5b")
            nc.gpsimd.dma_start(tC5b[:], dC5[:, 3:5])  # DT, P
            nc.gpsimd.dma_start(wts[1][:], dW[1])
            tBV = cpool.tile([D, RL], bf16, tag="BV")
            nc.sync.dma_start(tBV[:], dBV)
            tID = cpool.tile([D, D], bf16, tag="ID")
            nc.sync.dma_start(tID[:], dID)
            tOH = cpool.tile([L, 2, 2], bf16, tag="OH")
            nc.sync.dma_start(tOH[:], dOH)

            # one psum tile = all 8 banks; pair jj uses banks 4*(jj%2)..+4
            pp = ppool.tile([2, 8, 512], mybir.dt.float32, tag="ps", name="pp")
            ppv = lambda jj, c: pp[:, 4 * (jj % 2) + c, : BC * D].rearrange(
                "p (b d) -> p b d", b=BC
            )

            dOUTt = dOUT.transpose([1, 0, 2])  # [RL, B, D]

            S5 = lambda k: (L, k, 2, B, D)
            dists = [None, None]
            ebfs = {}

            def issue_dist_exp(jj):
                ebf = wpool.tile([L, 2, B, D], bf16, tag="ebf", name=f"ebf{jj}", bufs=2)
                ebfs[jj] = ebf
                for rr in range(2):
                    j = 2 * jj + rr
                    dist = wpool.tile(
                        [L, B, D], f32, tag="dist", name=f"dist{j}", bufs=2
                    )
                    nc.scalar.activation(
                        dist[:], tT[:], AF.Abs, bias=tRA[:, 0, j : j + 1]
                    )
                    nc.scalar.activation(
                        ebf[:, rr], dist[:], AF.Exp, scale=tRA[:, 1, j : j + 1]
                    )

            issue_dist_exp(0)

            for jj in range(NP):
                wt = wts[jj]
                last = jj == NP - 1
                if jj + 2 < NP:
                    nc.sync.dma_start(wts[jj + 2][:], dW[jj + 2])
                if jj + 1 < NP:
                    issue_dist_exp(jj + 1)
                ebf = ebfs.pop(jj)

                # psum groups open: bias = 128*b_v via identity-rhs matmul
                for c in range(NCH):
                    nc.tensor.matmul(
                        ppv(jj, c),
                        tBV[:, 2 * jj : 2 * jj + 2],
                        tID[:, None, :].to_broadcast((D, BC, D)),
                        start=True,
                        stop=False,
                    )

                # ---- DVE: 11-13 instructions, 12 element-passes
                q = wpool.tile([L, 2, B, D], bf16, tag="q", bufs=2)
                t = wpool.tile([L, 2, B, D], bf16, tag="t", bufs=2)
                a2a = wpool.tile([L, 2, 2, B, D], bf16, tag="a2a", bufs=1)
                a2b = wpool.tile([L, 2, 2, B, D], bf16, tag="a2b", bufs=1)
                qop = lambda: nc.vector.tensor_mul(
                    q[:],
                    tXP[:].to_broadcast((L, 2, B, D)),
                    wt[:, 0, :, :, :].to_broadcast((L, 2, B, D)),
                )
                aop = lambda ca, wk: nc.vector.tensor_mul(
                    ca[:],
                    (tC5a if wk == 1 else tC5b)[:, :, None].to_broadcast(S5(2)),
                    wt[:, wk : wk + 2].to_broadcast(S5(2)),
                )
                top = lambda rr: nc.vector.tensor_mul(t[:, rr], q[:, rr], ebf[:, rr])
                if jj == 0:
                    # pair 0: order by DMA/ACT arrival so the in-order DVE
                    # queue never blocks: q(XP+W0), a2a(C5a), t(exp), a2b(C5b)
                    qop(), aop(a2a, 1), top(0), top(1), aop(a2b, 3)
                else:
                    qop(), top(0), top(1), aop(a2a, 1), aop(a2b, 3)
                s12 = wpool.tile([L, 2, 2, B, D], bf16, tag="s12", bufs=1)
                nc.vector.tensor_add(s12[:], a2a[:], a2b[:])
                s3 = wpool.tile([L, 2, B, D], bf16, tag="s3", bufs=2)
                nc.vector.tensor_add(s3[:], s12[:, 0], s12[:, 1])
                tb = wpool.tile([L, 2, B, D], bf16, tag="tb", bufs=2)
                nc.vector.tensor_add(
                    tb[:], t[:], wt[:, 5].to_broadcast((L, 2, B, D))
                )
                p = wpool.tile([L, 2, B, D], bf16, tag="p", bufs=2)
                h = wpool.tile([L, 2, B, D], bf16, tag="h", bufs=2)
                wh = wpool.tile([L, 2, B, D], bf16, tag="wh", bufs=2)
                rsl = [slice(0, 2)] if not last else [slice(0, 1), slice(1, 2)]
                for rs in rsl:
                    # last pair: split the p->relu->wh tail per r so ACT/PE
                    # overlap DVE instead of serializing after it
                    nc.vector.tensor_add(p[:, rs], s3[:, rs], tb[:, rs])
                    nc.scalar.activation(h[:, rs], p[:, rs], AF.Relu)
                for rr in range(2):
                    nc.vector.tensor_mul(
                        wh[:, rr],
                        h[:, rr],
                        wt[:, 6, rr].to_broadcast((L, B, D)),
                    )
                    for c in range(NCH):
                        nc.tensor.matmul(
                            ppv(jj, c),
                            tOH[:, rr],
                            wh[:, rr, c * BC : (c + 1) * BC, :],
                            start=False,
                            stop=(rr == 1),
                        )

                # epilogue for this pair: relu(psum) -> sbuf, DMA out
                outf = opool.tile(
                    [2, B, D], mybir.dt.float32, tag="outf", name=f"outf{jj}", bufs=2
                )
                h4 = 4 * (jj % 2)
                nc.scalar.activation(
                    outf.rearrange("p (c b) d -> p c b d", c=NCH),
                    pp[:, h4 : h4 + 4, : BC * D].rearrange(
                        "p c (b d) -> p c b d", b=BC
                    ),
                    AF.Relu,
                )
                nc.sync.dma_start(dOUTt[2 * jj : 2 * jj + 2], outf[:])

    nc.compile()
    return nc


def _prep(X, T, M, DT, P, alpha, w_t, b_t, w_v, b_v):
    """Host-side shard prep: returns in_maps for the 8 cores."""
    X, T, M, DT, P, alpha, w_t, b_t, w_v, b_v = (
        np.asarray(a) for a in (X, T, M, DT, P, alpha, w_t, b_t, w_v, b_v)
    )
    refs = np.linspace(INIT_TIME, MAX_TS, R, dtype=np.float32)
    arelu = np.maximum(alpha.reshape(R).astype(np.float32), 0.0)

    Tt = np.ascontiguousarray(T.transpose(1, 0, 2)).astype(np.float32)
    Xb = X.transpose(1, 0, 2).astype(BF16)
    c5 = np.ascontiguousarray(
        np.stack(
            [
                np.maximum(Xb, 0),
                Xb,
                M.transpose(1, 0, 2).astype(BF16),
                DT.transpose(1, 0, 2).astype(BF16),
                P.transpose(1, 0, 2).astype(BF16),
            ],
            axis=1,
        )
    )  # [L, 5, B, D]
    id48 = np.eye(D, dtype=np.float32).astype(BF16)
    ohp = np.zeros((L, 2, 2), dtype=np.float32)
    ohp[:, 0, 0] = 1.0
    ohp[:, 1, 1] = 1.0
    ohp = ohp.astype(BF16)

    # W[pair, l, k, rr, 1, d]: channels (w1, w0, w2, w3, w4, 5*b_t, w_v)
    wk_full = np.concatenate(
        [
            w_t[..., 1:2],
            w_t[..., 0:1],
            w_t[..., 2:5],
            5.0 * b_t,
            w_v[..., None],
        ],
        axis=3,
    )  # [R, L, D, 7]
    in_maps = []
    for i in range(8):
        r0 = i * RL
        wx = wk_full[r0 : r0 + RL].transpose(1, 3, 0, 2)  # [L, 7, RL, D]
        wx = wx.reshape(L, 7, NP, 2, D).transpose(2, 0, 1, 3, 4)  # [NP, L, 7, 2, D]
        wx = np.ascontiguousarray(wx[:, :, :, :, None, :]).astype(BF16)
        ra = np.broadcast_to(
            np.stack([-refs[r0 : r0 + RL], -arelu[r0 : r0 + RL]]), (L, 2, RL)
        ).astype(np.float32)
        bvl = np.ascontiguousarray(
            (128.0 * b_v[r0 : r0 + RL, 0, :]).T
        ).astype(BF16)  # [D, RL]
        in_maps.append(
            {
                "Tt": Tt,
                "C5": c5,
                "W": wx,
                "RA": np.ascontiguousarray(ra),
                "BVl": bvl,
                "ID48": id48,
                "OHP": ohp,
            }
        )
    return in_maps


def run(trace=False, **inputs):
    if "nc" not in _CACHE:
        _CACHE["nc"] = _build()
    nc = _CACHE["nc"]
    in_maps = _prep(**inputs)
    res = run_bass_kernel_spmd(nc, in_maps, core_ids=list(range(8)), trace=trace)
    out = np.empty((B, R, D), dtype=np.float32)
    for i in range(8):
        out[:, i * RL : (i + 1) * RL, :] = res.results[i]["out"]
    return out, res


def kernel(**inputs) -> np.ndarray:
    out, _ = run(trace=False, **inputs)
    return out


# revision 18
# speedup vs baseline: 1.3039x; 1.0105x over previous
"""ALNN layer on 8 TRN2 NeuronCores (Bass/Tile, SPMD — no collectives).

Math (per reference):
  ref_r = linspace(0, 48, 64);  a_r = relu(alpha_r)
  e[b,r,l,d]  = exp(-a_r * |T[b,l,d] - ref_r|)
  p[b,r,l,d]  = w0*X + w1*relu(X)*e + w2*M + w3*DT + w4*P + 5*b_t[r,l,d]
  h           = relu(p)
  out[b,r,d]  = relu( sum_l w_v[r,l,d]*h + 128*b_v[r,d] )

Design v2 (DVE-roofline focused):
- Shard R=64 across the 8 cores (8 r each); inputs replicated; host
  concatenates the per-core [B, 8, D] outputs. No cross-core traffic.
- Layout: partition = L (=128), free = (r-pair, b, d). Per pair the 12
  irreducible DVE element-passes are packed into 11 instructions, with the
  4 channel muls done as two 2-channel-wide ops (bf16 2x_1p mode) reading a
  host-packed C5 = [XP, X, M, DT, P] tile, and the add tree packed as
  [aXM]+[aDP] -> s12, s12[0]+s12[1] -> s3.
- DMA is ~12 large transfers ordered so ACT (T, RN, AN) and DVE (XP, W0)
  start by ~4us; weight packs are one DMA per pair (prefetched bufs=2).
- ACT (ScalarE): per-r dist=Abs(T-ref), e=Exp(-a*dist) (f32 dist), relu(p),
  and the per-pair psum epilogue relu. Pair jj+1's dist/exp are issued
  before pair jj's relu so the ACT queue never blocks DVE.
- TensorE: per (pair, b-chunk) PSUM accumulation groups (4 banks per pair,
  2 pairs in flight over the 8 banks): bias open via identity-rhs matmul,
  then per-r ones-column lhsT matmuls sum wv*h over l. Each pair's group
  closes right after its wh, so relu(psum) + output DMA overlap the next
  pair's compute (no serial tail).
"""
import sys

import numpy as np

if "/opt/trn_rl_repo" not in sys.path:
    sys.path.insert(0, "/opt/trn_rl_repo")

import ml_dtypes

from concourse import bacc, mybir
import concourse.tile as tile
from concourse.bass_utils import run_bass_kernel_spmd

BF16 = ml_dtypes.bfloat16
B, L, D = 32, 128, 48
R = 64
RL = R // 8  # r per core
NP = RL // 2  # r-pairs per core
INIT_TIME, MAX_TS = 0.0, 48.0

_CACHE = {}


def _build():
    nc = bacc.Bacc("TRN2", target_bir_lowering=False, debug=False, num_devices=8)
    f32, bf16 = mybir.dt.float32, mybir.dt.bfloat16
    AF = mybir.ActivationFunctionType

    # DRAM parameters (per-core shards / replicas)
    dTt = nc.dram_tensor("Tt", [L, B, D], f32, kind="ExternalInput").ap()
    # C5 channels: (XP, X, M, DT, P)
    dC5 = nc.dram_tensor("C5", [L, 5, B, D], bf16, kind="ExternalInput").ap()
    # W channels: (w1, w0, w2, w3, w4, 5*b_t, w_v) per r-pair
    dW = nc.dram_tensor("W", [NP, L, 7, 2, 1, D], bf16, kind="ExternalInput").ap()
    # RA[:, 0] = -refs (dist bias), RA[:, 1] = -relu(alpha) (exp scale)
    dRA = nc.dram_tensor("RA", [L, 2, RL], f32, kind="ExternalInput").ap()
    dBV = nc.dram_tensor("BVl", [D, RL], bf16, kind="ExternalInput").ap()
    dID = nc.dram_tensor("ID48", [D, D], bf16, kind="ExternalInput").ap()
    dOH = nc.dram_tensor("OHP", [L, 2, 2], bf16, kind="ExternalInput").ap()
    dOUT = nc.dram_tensor("out", [B, RL, D], f32, kind="ExternalOutput").ap()

    NCH = 4  # psum b-chunks per pair (2*8*48 = 768 f32 < ... 8*48=384/bank)
    BC = B // NCH  # 8 b per chunk

    with tile.TileContext(nc) as tc:
        with (
            tc.tile_pool(name="const", bufs=1) as cpool,
            tc.tile_pool(name="work", bufs=2) as wpool,
            tc.tile_pool(name="psum", bufs=1, space="PSUM") as ppool,
            tc.tile_pool(name="outp", bufs=1) as opool,
        ):
            # ---- DMA startup plan: two queues in parallel, exact-dep tiles.
            # sync: ACT-chain gates (RA, T) then matmul consts + W2/W3.
            # gpsimd: DVE gates (W0, XP, C5a, C5b, W1).
            tRA = cpool.tile([L, 2, RL], f32, tag="RA")
            nc.sync.dma_start(tRA[:], dRA)
            tT = cpool.tile([L, B, D], f32, tag="T")
            nc.sync.dma_start(tT[:], dTt)
            wts = [
                wpool.tile([L, 7, 2, 1, D], bf16, tag="wt", name=f"wt{j}", bufs=2)
                for j in range(NP)
            ]
            nc.gpsimd.dma_start(wts[0][:], dW[0])
            tC5 = cpool.tile([L, 5, B, D], bf16, tag="C5")
            nc.gpsimd.dma_start(tC5[:, 0:1], dC5[:, 0:1])  # XP: gates q-mul
            nc.gpsimd.dma_start(tC5[:, 1:3], dC5[:, 1:3])  # X, M: gate mul2a
            nc.gpsimd.dma_start(tC5[:, 3:5], dC5[:, 3:5])  # DT, P
            nc.gpsimd.dma_start(wts[1][:], dW[1])
            tBV = cpool.tile([D, RL], bf16, tag="BV")
            nc.sync.dma_start(tBV[:], dBV)
            tID = cpool.tile([D, D], bf16, tag="ID")
            nc.sync.dma_start(tID[:], dID)
            tOH = cpool.tile([L, 2, 2], bf16, tag="OH")
            nc.sync.dma_start(tOH[:], dOH)

            # one psum tile = all 8 banks; pair jj uses banks 4*(jj%2)..+4
            pp = ppool.tile([2, 8, 512], mybir.dt.float32, tag="ps", name="pp")
            ppv = lambda jj, c: pp[:, 4 * (jj % 2) + c, : BC * D].rearrange(
                "p (b d) -> p b d", b=BC
            )

            dOUTt = dOUT.transpose([1, 0, 2])  # [RL, B, D]

            S5 = lambda k: (L, k, 2, B, D)
            dists = [None, None]
            ebfs = {}

            def issue_dist_exp(jj):
                ebf = wpool.tile([L, 2, B, D], bf16, tag="ebf", name=f"ebf{jj}", bufs=2)
                ebfs[jj] = ebf
                for rr in range(2):
                    j = 2 * jj + rr
                    dist = wpool.tile(
                        [L, B, D], f32, tag="dist", name=f"dist{j}", bufs=2
                    )
                    nc.scalar.activation(
                        dist[:], tT[:], AF.Abs, bias=tRA[:, 0, j : j + 1]
                    )
                    nc.scalar.activation(
                        ebf[:, rr], dist[:], AF.Exp, scale=tRA[:, 1, j : j + 1]
                    )

            issue_dist_exp(0)

            for jj in range(NP):
                wt = wts[jj]
                last = jj == NP - 1
                if jj + 2 < NP:
                    nc.sync.dma_start(wts[jj + 2][:], dW[jj + 2])
                if jj + 1 < NP:
                    issue_dist_exp(jj + 1)
                ebf = ebfs.pop(jj)

                # psum groups open: bias = 128*b_v via identity-rhs matmul
                for c in range(NCH):
                    nc.tensor.matmul(
                        ppv(jj, c),
                        tBV[:, 2 * jj : 2 * jj + 2],
                        tID[:, None, :].to_broadcast((D, BC, D)),
                        start=True,
                        stop=False,
                    )

                # ---- DVE: 8-11 instructions, 12 element-passes
                t = wpool.tile([L, 2, B, D], bf16, tag="t", bufs=2)
                a5 = wpool.tile([L, 5, 2, B, D], bf16, tag="a5", bufs=1)
                if jj == 0:
                    # pair 0: split the channel mul and order by DMA/ACT
                    # arrival so the in-order DVE queue never blocks:
                    # q(XP+W0), a2a(C5a), t(exp chain), a2b(C5b)
                    nc.vector.tensor_mul(
                        a5[:, 0],
                        tC5[:, 0:1].to_broadcast((L, 2, B, D)),
                        wt[:, 0, :, :, :].to_broadcast((L, 2, B, D)),
                    )
                    nc.vector.tensor_mul(
                        a5[:, 1:3],
                        tC5[:, 1:3, None].to_broadcast(S5(2)),
                        wt[:, 1:3].to_broadcast(S5(2)),
                    )
                    nc.vector.tensor_mul(t[:, 0], a5[:, 0, 0], ebf[:, 0])
                    nc.vector.tensor_mul(t[:, 1], a5[:, 0, 1], ebf[:, 1])
                    nc.vector.tensor_mul(
                        a5[:, 3:5],
                        tC5[:, 3:5, None].to_broadcast(S5(2)),
                        wt[:, 3:5].to_broadcast(S5(2)),
                    )
                else:
                    # steady state: all inputs resident — one 5-channel mul
                    nc.vector.tensor_mul(
                        a5[:],
                        tC5[:, :, None].to_broadcast(S5(5)),
                        wt[:, 0:5].to_broadcast(S5(5)),
                    )
                    nc.vector.tensor_mul(t[:], a5[:, 0], ebf[:])
                s12 = wpool.tile([L, 2, 2, B, D], bf16, tag="s12", bufs=1)
                nc.vector.tensor_add(s12[:], a5[:, 1:3], a5[:, 3:5])
                s3 = wpool.tile([L, 2, B, D], bf16, tag="s3", bufs=2)
                nc.vector.tensor_add(s3[:], s12[:, 0], s12[:, 1])
                tb = wpool.tile([L, 2, B, D], bf16, tag="tb", bufs=2)
                nc.vector.tensor_add(
                    tb[:], t[:], wt[:, 5].to_broadcast((L, 2, B, D))
                )
                p = wpool.tile([L, 2, B, D], bf16, tag="p", bufs=2)
                h = wpool.tile([L, 2, B, D], bf16, tag="h", bufs=2)
                wh = wpool.tile([L, 2, B, D], bf16, tag="wh", bufs=2)
                if not last:
                    nc.vector.tensor_add(p[:], s3[:], tb[:])
                    nc.scalar.activation(h[:], p[:], AF.Relu)
                else:
                    # last pair: keep the whole tail on DVE (relu via 4x-mode
                    # tensor_scalar max) and split per r, so no cross-engine
                    # round-trip is exposed at the end of the kernel
                    for rs in (slice(0, 1), slice(1, 2)):
                        nc.vector.tensor_add(p[:, rs], s3[:, rs], tb[:, rs])
                        nc.vector.tensor_scalar_max(h[:, rs], p[:, rs], 0.0)
                for rr in range(2):
                    nc.vector.tensor_mul(
                        wh[:, rr],
                        h[:, rr],
                        wt[:, 6, rr].to_broadcast((L, B, D)),
                    )
                    for c in range(NCH):
                        nc.tensor.matmul(
                            ppv(jj, c),
                            tOH[:, rr],
                            wh[:, rr, c * BC : (c + 1) * BC, :],
                            start=False,
                            stop=(rr == 1),
                        )

                # epilogue for this pair: relu(psum) -> sbuf, DMA out
                outf = opool.tile(
                    [2, B, D], mybir.dt.float32, tag="outf", name=f"outf{jj}", bufs=2
                )
                h4 = 4 * (jj % 2)
                nc.scalar.activation(
                    outf.rearrange("p (c b) d -> p c b d", c=NCH),
                    pp[:, h4 : h4 + 4, : BC * D].rearrange(
                        "p c (b d) -> p c b d", b=BC
                    ),
                    AF.Relu,
                )
                nc.sync.dma_start(dOUTt[2 * jj : 2 * jj + 2], outf[:])

    nc.compile()
    return nc


def _prep(X, T, M, DT, P, alpha, w_t, b_t, w_v, b_v):
    """Host-side shard prep: returns in_maps for the 8 cores."""
    X, T, M, DT, P, alpha, w_t, b_t, w_v, b_v = (
        np.asarray(a) for a in (X, T, M, DT, P, alpha, w_t, b_t, w_v, b_v)
    )
    refs = np.linspace(INIT_TIME, MAX_TS, R, dtype=np.float32)
    arelu = np.maximum(alpha.reshape(R).astype(np.float32), 0.0)

    Tt = np.ascontiguousarray(T.transpose(1, 0, 2)).astype(np.float32)
    Xb = X.transpose(1, 0, 2).astype(BF16)
    c5 = np.ascontiguousarray(
        np.stack(
            [
                np.maximum(Xb, 0),
                Xb,
                M.transpose(1, 0, 2).astype(BF16),
                DT.transpose(1, 0, 2).astype(BF16),
                P.transpose(1, 0, 2).astype(BF16),
            ],
            axis=1,
        )
    )  # [L, 5, B, D]
    id48 = np.eye(D, dtype=np.float32).astype(BF16)
    ohp = np.zeros((L, 2, 2), dtype=np.float32)
    ohp[:, 0, 0] = 1.0
    ohp[:, 1, 1] = 1.0
    ohp = ohp.astype(BF16)

    # W[pair, l, k, rr, 1, d]: channels (w1, w0, w2, w3, w4, 5*b_t, w_v)
    wk_full = np.concatenate(
        [
            w_t[..., 1:2],
            w_t[..., 0:1],
            w_t[..., 2:5],
            5.0 * b_t,
            w_v[..., None],
        ],
        axis=3,
    )  # [R, L, D, 7]
    in_maps = []
    for i in range(8):
        r0 = i * RL
        wx = wk_full[r0 : r0 + RL].transpose(1, 3, 0, 2)  # [L, 7, RL, D]
        wx = wx.reshape(L, 7, NP, 2, D).transpose(2, 0, 1, 3, 4)  # [NP, L, 7, 2, D]
        wx = np.ascontiguousarray(wx[:, :, :, :, None, :]).astype(BF16)
        ra = np.broadcast_to(
            np.stack([-refs[r0 : r0 + RL], -arelu[r0 : r0 + RL]]), (L, 2, RL)
        ).astype(np.float32)
        bvl = np.ascontiguousarray(
            (128.0 * b_v[r0 : r0 + RL, 0, :]).T
        ).astype(BF16)  # [D, RL]
        in_maps.append(
            {
                "Tt": Tt,
                "C5": c5,
                "W": wx,
                "RA": np.ascontiguousarray(ra),
                "BVl": bvl,
                "ID48": id48,
                "OHP": ohp,
            }
        )
    return in_maps


def run(trace=False, **inputs):
    if "nc" not in _CACHE:
        _CACHE["nc"] = _build()
    nc = _CACHE["nc"]
    in_maps = _prep(**inputs)
    res = run_bass_kernel_spmd(nc, in_maps, core_ids=list(range(8)), trace=trace)
    out = np.empty((B, R, D), dtype=np.float32)
    for i in range(8):
        out[:, i * RL : (i + 1) * RL, :] = res.results[i]["out"]
    return out, res


def kernel(**inputs) -> np.ndarray:
    out, _ = run(trace=False, **inputs)
    return out
